# revision 4
# baseline (speedup 1.0000x reference)
"""Multi-head attention kernel for 8 Trainium2 NeuronCores (v2 schedule).

Same math/layout as v1: batch x head-halves sharding, scores_T [k,q] per
head pair via row-packed PE matmuls, exp on ScalarE (the critical engine,
971ns per [128,1024] tile), U via col-packed matmuls, quad-packed ones-
matmul sumexp, DVE/gpsimd normalize, row-sharded output projection.

v2 scheduling: ScalarE streams one exp per slot; every other PE task is
broken into <=2-matmul chunks fed by a budgeted per-slot driver so the PE
never lumps >~750ns between consecutive scores emissions (v1 dumped whole
9-matmul groups into single slots, starving ScalarE for ~120us). U matmuls
are decoupled from their exp by a deep et ring (NB tiles) and emitted only
once their V tile exists; projection/outproj groups flow through the same
driver with deadlines. Output projection of the last q-block is split by
head-pair so only half of it remains after the final exp.
"""

import sys

sys.path.insert(0, "/opt/trn_rl_repo")

import numpy as np
import ml_dtypes

import concourse.bass as bass
import concourse.bacc as bacc
import concourse.mybir as mybir
import concourse.tile as tile
from concourse.bass_utils import run_bass_kernel_spmd

BF16 = mybir.dt.bfloat16
F32 = mybir.dt.float32
NPBF16 = ml_dtypes.bfloat16

B, S, E = 4, 2048, 1024
H_LOC = 8
D = 64
OL = H_LOC * D     # 512
N_CORES = 8
QB = 512
NQB = S // QB      # 4
NKT = S // 128     # 16
NET = E // 128     # 8
NB = 20            # et ring depth (deferred-U decoupling)


def build_program():
    from contextlib import ExitStack

    nc = bacc.Bacc("TRN2", debug=False, num_devices=N_CORES)

    xT = nc.dram_tensor("xT", [E, S], BF16, kind="ExternalInput")
    wqT = nc.dram_tensor("wqT", [E, OL], BF16, kind="ExternalInput")
    wkT = nc.dram_tensor("wkT", [E, OL], BF16, kind="ExternalInput")
    wvT = nc.dram_tensor("wvT", [E, OL], BF16, kind="ExternalInput")
    woT = nc.dram_tensor("woT", [OL, E], BF16, kind="ExternalInput")
    bqc = nc.dram_tensor("bqc", [128, 4], F32, kind="ExternalInput")
    bkc = nc.dram_tensor("bkc", [128, 4], F32, kind="ExternalInput")
    yT = nc.dram_tensor("yT", [E, S], F32, kind="ExternalOutput")

    with tile.TileContext(nc) as tc, ExitStack() as est:
        xt_p = est.enter_context(tc.tile_pool(name="xt", bufs=NET))
        wq_p = est.enter_context(tc.tile_pool(name="wq", bufs=NET))
        wk_p = est.enter_context(tc.tile_pool(name="wk", bufs=NET))
        wv_p = est.enter_context(tc.tile_pool(name="wv", bufs=NET))
        wo_p = est.enter_context(tc.tile_pool(name="wo", bufs=4))
        bias_p = est.enter_context(tc.tile_pool(name="bias", bufs=4))
        qt_p = est.enter_context(tc.tile_pool(name="qt", bufs=4))
        kt_p = est.enter_context(tc.tile_pool(name="kt", bufs=4))
        vb_p = est.enter_context(tc.tile_pool(name="vb", bufs=NKT))
        pj_p = est.enter_context(tc.tile_pool(name="pj", bufs=1, space="PSUM"))
        sc_p = est.enter_context(tc.tile_pool(name="sc", bufs=2, space="PSUM"))
        u_p = est.enter_context(tc.tile_pool(name="u", bufs=2, space="PSUM"))
        se_p = est.enter_context(tc.tile_pool(name="se", bufs=1, space="PSUM"))
        ex_p = est.enter_context(tc.tile_pool(name="ex", bufs=NB))
        at_p = est.enter_context(tc.tile_pool(name="at", bufs=16))
        nrm_p = est.enter_context(tc.tile_pool(name="nrm", bufs=2))
        ys_p = est.enter_context(tc.tile_pool(name="ys", bufs=2))
        yp_p = est.enter_context(tc.tile_pool(name="yp", bufs=8))
        usb_p = est.enter_context(tc.tile_pool(name="usb", bufs=4))

        # ---- activation warm + input DMAs (column-chunked: tblock0 first)
        warm = bias_p.tile([1, 16], F32, tag="warm")
        nc.vector.memset(warm[:], 0.0)
        warm2 = bias_p.tile([1, 16], F32, tag="warm2")
        nc.scalar.activation(warm2[:], warm[:],
                             mybir.ActivationFunctionType.Exp)

        xts = [xt_p.tile([128, S], BF16, tag="xt", name="xt")
               for _ in range(NET)]
        wts = {
            name: [pool.tile([128, OL], BF16, tag="w" + name,
                             name="w" + name) for _ in range(NET)]
            for name, pool in (("q", wq_p), ("k", wk_p), ("v", wv_p))
        }
        # prologue critical path first: wq/wk o-tile 0 slivers + x tblock 0
        for e in range(NET):
            nc.sync.dma_start(wts["q"][e][:, 0:128],
                              wqT[e * 128:(e + 1) * 128, 0:128])
            nc.sync.dma_start(wts["k"][e][:, 0:128],
                              wkT[e * 128:(e + 1) * 128, 0:128])
        for e in range(NET):
            nc.sync.dma_start(xts[e][:, 0:QB], xT[e * 128:(e + 1) * 128, 0:QB])
        for e in range(NET):
            nc.sync.dma_start(wts["q"][e][:, 128:OL],
                              wqT[e * 128:(e + 1) * 128, 128:OL])
            nc.sync.dma_start(wts["k"][e][:, 128:OL],
                              wkT[e * 128:(e + 1) * 128, 128:OL])
        for e in range(NET):
            nc.sync.dma_start(wts["v"][e][:], wvT[e * 128:(e + 1) * 128, :])
        for j in range(1, NQB):
            for e in range(NET):
                nc.sync.dma_start(xts[e][:, j * QB:(j + 1) * QB],
                                  xT[e * 128:(e + 1) * 128, j * QB:(j + 1) * QB])
        wos = [wo_p.tile([128, E], BF16, tag="wo", name="wo")
               for _ in range(4)]
        for p in range(4):
            nc.sync.dma_start(wos[p][:], woT[p * 128:(p + 1) * 128, :])
        bqs = bias_p.tile([128, 4], F32, tag="bqc")
        bks = bias_p.tile([128, 4], F32, tag="bkc")
        onecol = bias_p.tile([128, 1], BF16, tag="onecol")
        nc.sync.dma_start(bqs[:], bqc[:])
        nc.sync.dma_start(bks[:], bkc[:])
        nc.vector.memset(onecol[:], 1.0)

        qts = [qt_p.tile([128, S], BF16, tag="qt", name="qt")
               for _ in range(4)]
        kts = [kt_p.tile([128, S], BF16, tag="kt", name="kt")
               for _ in range(4)]
        vbs = [vb_p.tile([128, OL], BF16, tag="vb", name="vb")
               for _ in range(NKT)]
        atts = [[at_p.tile([128, QB], BF16, tag="at", name="at")
                 for _ in range(4)] for _ in range(NQB)]
        yps = [yp_p.tile([128, QB], BF16, tag="yp", name="yp")
               for _ in range(NET)]

        # ---- filler chunk machinery ----------------------------------
        # A chunk emits <=2 (occasionally 3) full matmuls. Chunks of one
        # group run in order; the driver never interleaves two groups.

        def qk_chunks(i, j, which):
            """Q or K projection o-tile i, t-block j: 4 chunks of 2 MMs."""
            w = wts[which]
            bias_t = bqs if which == "q" else bks
            dest = qts[i] if which == "q" else kts[i]
            state = {}

            def chunk(c):
                def emit():
                    if c == 0:
                        state["acc"] = pj_p.tile([128, QB], F32, tag="pj",
                                                 name="pj")
                    acc = state["acc"]
                    for e in (2 * c, 2 * c + 1):
                        nc.tensor.matmul(
                            acc[:],
                            w[e][:, i * 128:(i + 1) * 128],
                            xts[e][:, j * QB:(j + 1) * QB],
                            start=(e == 0), stop=(e == NET - 1),
                        )
                    if c == 3:
                        nc.vector.tensor_scalar_add(
                            dest[:, j * QB:(j + 1) * QB], acc[:],
                            bias_t[:, i:i + 1])
                return emit
            return [chunk(c) for c in range(4)]

        def v_chunks(ti):
            """V projection for k-tile ti: 4 chunks (3,2,2,2 MMs)."""
            state = {}

            def chunk(c):
                def emit():
                    if c == 0:
                        state["acc"] = pj_p.tile([128, OL], F32, tag="pj",
                                                 name="pjv")
                    acc = state["acc"]
                    for e in (2 * c, 2 * c + 1):
                        nc.tensor.matmul(
                            acc[:],
                            xts[e][:, ti * 128:(ti + 1) * 128],
                            wts["v"][e][:],
                            start=(e == 0), stop=(e == NET - 1),
                        )
                    if c == 3:
                        nc.vector.tensor_copy(vbs[ti][:], acc[:])
                return emit
            return [chunk(c) for c in range(4)]

        def og_chunks(qb, eo):
            """Full output projection (4 pair-tiles): 2 chunks of 2 MMs."""
            state = {}

            def chunk(c):
                def emit():
                    if c == 0:
                        state["y"] = pj_p.tile([128, QB], F32, tag="pj",
                                               name="y")
                    y = state["y"]
                    for p2 in (2 * c, 2 * c + 1):
                        nc.tensor.matmul(
                            y[:],
                            wos[p2][:, eo * 128:(eo + 1) * 128],
                            atts[qb][p2][:],
                            start=(p2 == 0), stop=(p2 == 3),
                        )
                    if c == 1:
                        ysb = ys_p.tile([128, QB], F32, tag="ys", name="ys")
                        nc.vector.tensor_copy(ysb[:], y[:])
                        nc.sync.dma_start(
                            yT[eo * 128:(eo + 1) * 128,
                               qb * QB:(qb + 1) * QB], ysb[:])
                return emit
            return [chunk(c) for c in range(2)]

        def og3a_chunk(eo):
            """Last q-block outproj, pairs 0/1 only -> SBUF partial."""
            def emit():
                y = pj_p.tile([128, QB], F32, tag="pj", name="y3a")
                for p2 in (0, 1):
                    nc.tensor.matmul(
                        y[:],
                        wos[p2][:, eo * 128:(eo + 1) * 128],
                        atts[3][p2][:],
                        start=(p2 == 0), stop=(p2 == 1),
                    )
                nc.vector.tensor_copy(yps[eo][:], y[:])
            return emit

        def og3b_chunk(eo):
            """Last q-block outproj, pairs 2/3 + add partial + DMA."""
            def emit():
                y = pj_p.tile([128, QB], F32, tag="pj", name="y3b")
                for p2 in (2, 3):
                    nc.tensor.matmul(
                        y[:],
                        wos[p2][:, eo * 128:(eo + 1) * 128],
                        atts[3][p2][:],
                        start=(p2 == 2), stop=(p2 == 3),
                    )
                ysb = ys_p.tile([128, QB], F32, tag="ys", name="ys")
                nc.vector.tensor_add(ysb[:], y[:], yps[eo][:])
                nc.sync.dma_start(
                    yT[eo * 128:(eo + 1) * 128, 3 * QB:4 * QB], ysb[:])
            return emit

        # ---- filler queue with deadlines ------------------------------
        # entries: (deadline_slot or None, vkt or None, [chunks]); groups
        # appear in queue order; deadline = slot by which the LAST chunk
        # must have been emitted.
        def sidx(quad, qb, pi, kt):
            return ((quad * NQB + qb) * 2 + pi) * NKT + kt

        filler = []
        for j in range(1, 4):                       # K(0,j) by slot 4j-1
            filler.append([4 * j - 1, None, qk_chunks(0, j, "k")])
        filler.append([15, None, qk_chunks(1, 0, "k")])
        filler.append([15, None, qk_chunks(1, 0, "q")])
        for j in range(1, 4):                       # K(1,j) by 16+4j-1
            filler.append([16 + 4 * j - 1, None, qk_chunks(1, j, "k")])
        for t in range(NKT):                        # V(t): U(t) ring-forced
            filler.append([14 + t, t, v_chunks(t)])
        filler.append([sidx(0, 1, 0, 0) - 1, None, qk_chunks(0, 1, "q")])
        filler.append([sidx(0, 1, 1, 0) - 1, None, qk_chunks(1, 1, "q")])
        filler.append([sidx(0, 2, 0, 0) - 1, None, qk_chunks(0, 2, "q")])
        filler.append([sidx(0, 2, 1, 0) - 1, None, qk_chunks(1, 2, "q")])
        filler.append([sidx(0, 3, 0, 0) - 1, None, qk_chunks(0, 3, "q")])
        filler.append([sidx(0, 3, 1, 0) - 1, None, qk_chunks(1, 3, "q")])
        # quad1 K/Q: needed from slot 128 on
        for i in (2, 3):
            for j in range(4):
                filler.append([128 + (4 * (i - 2) + j) * 8, None,
                               qk_chunks(i, j, "k")])
        filler.append([sidx(1, 1, 0, 0) - 1, None, qk_chunks(2, 0, "q")])
        filler.append([sidx(1, 1, 1, 0) - 1, None, qk_chunks(3, 0, "q")])
        filler.append([sidx(1, 1, 1, 8) - 1, None, qk_chunks(2, 1, "q")])
        filler.append([sidx(1, 2, 0, 0) - 1, None, qk_chunks(3, 1, "q")])
        filler.append([sidx(1, 2, 0, 8) - 1, None, qk_chunks(2, 2, "q")])
        filler.append([sidx(1, 2, 1, 0) - 1, None, qk_chunks(3, 2, "q")])
        filler.append([sidx(1, 2, 1, 8) - 1, None, qk_chunks(2, 3, "q")])
        filler.append([sidx(1, 3, 0, 0) - 1, None, qk_chunks(3, 3, "q")])
        # outproj fillers: og(qb) after block (1,qb); og3a after (0,3)
        for eo in range(NET):
            filler.append([None, None, [og3a_chunk(eo)],
                           sidx(0, 3, 1, NKT - 1) + 1,
                           sidx(0, 3, 1, NKT - 1) + 1])
        for qb in range(3):
            for eo in range(NET):
                filler.append([None, None, og_chunks(qb, eo),
                               sidx(1, qb, 1, NKT - 1) + 1,
                               sidx(1, qb, 1, NKT - 1) + 1])
        for f in filler:
            if len(f) == 3:
                f.extend([0, 0])   # earliest slot, u_req
        dl_part = [f for f in filler if f[0] is not None]
        og_part = [f for f in filler if f[0] is None]
        dl_part.sort(key=lambda f: f[0])
        filler = dl_part + og_part

        # ---- prologue: Q(0,0), K(0,0) whole groups -------------------
        for which in ("q", "k"):
            for ch in qk_chunks(0, 0, which):
                ch()

        # ---- attention slot loop -------------------------------------
        step_list = []
        for quad in range(2):
            for qb in range(NQB):
                for pi in range(2):
                    for kt in range(NKT):
                        step_list.append((quad, qb, 2 * quad + pi, pi, kt))
        nsteps = len(step_list)

        def emit_scores(quad, qb, pair, kt):
            sc = sc_p.tile([128, 2 * QB], F32, tag="sc", name="sc")
            nc.tensor.matmul(
                sc[:, 0:QB],
                kts[pair][0:64, kt * 128:(kt + 1) * 128],
                qts[pair][0:64, qb * QB:(qb + 1) * QB],
                start=True, stop=True, tile_position=(0, 0),
            )
            nc.tensor.matmul(
                sc[:, QB:2 * QB],
                kts[pair][64:128, kt * 128:(kt + 1) * 128],
                qts[pair][64:128, qb * QB:(qb + 1) * QB],
                start=True, stop=True, tile_position=(64, 0),
            )
            return sc

        # deferred-U state
        cur = {}
        ets = [None] * nsteps
        vdone = set()        # kt with fully-emitted V group
        vemitted = 0

        def emit_u(j):
            quad, qb, pair, pi, kt = step_list[j]
            et = ets[j]
            if (pi, kt) == (0, 0):
                cur["uA"] = u_p.tile([128, QB], F32, tag="u", name="uA")
                cur["uB"] = u_p.tile([128, QB], F32, tag="u", name="uB")
                cur["seb"] = se_p.tile([128, QB], F32, tag="se", name="seb")
                cur["etA"] = [None] * NKT
            if pi == 0:
                cur["etA"][kt] = et
            u = cur["uA"] if pi == 0 else cur["uB"]
            for sub in range(2):
                hcol = (pair * 2 + sub) * D
                nc.tensor.matmul(
                    u[sub * 64:(sub + 1) * 64, :],
                    vbs[kt][:, hcol:hcol + D],
                    et[:, sub * QB:(sub + 1) * QB],
                    start=(kt == 0), stop=(kt == NKT - 1),
                    tile_position=(0, sub * 64),
                    skip_group_check=True,
                )
            if pi == 1 and kt == 0:
                ua_sb = usb_p.tile([128, QB], BF16, tag="usb", name="ua_sb")
                nc.vector.tensor_copy(ua_sb[:], cur["uA"][:])
                cur["uA_sb"] = ua_sb
            if pi == 1:
                seb = cur["seb"]
                epair = (cur["etA"][kt], et)
                for g in range(4):
                    nc.tensor.matmul(
                        seb[g * 32:g * 32 + 1, :],
                        onecol[:],
                        epair[g // 2][:, (g % 2) * QB:(g % 2 + 1) * QB],
                        start=(kt == 0), stop=(kt == NKT - 1),
                        tile_position=(0, g * 32),
                        skip_group_check=True,
                    )
                if kt == NKT - 1:
                    pA, pB = 2 * quad, 2 * quad + 1
                    ub_sb = usb_p.tile([128, QB], BF16, tag="usb",
                                       name="ub_sb")
                    nc.vector.tensor_copy(ub_sb[:], cur["uB"][:])
                    seb_t = cur["seb"]
                    qb_t = qb

                    def norm_sub(u2, pr, sub):
                        def emit():
                            g = (pr % 2) * 2 + sub
                            rcs = nrm_p.tile([1, QB], F32, tag="rcs",
                                             name="rcs")
                            nc.vector.tensor_copy(
                                rcs[:], seb_t[g * 32:g * 32 + 1, :])
                            rcr = nrm_p.tile([1, QB], F32, tag="rcr",
                                             name="rcr")
                            nc.vector.reciprocal_approx_fast(rcr[:], rcs[:])
                            bcf = nrm_p.tile([128, QB], F32, tag="bcf",
                                             name="bcf")
                            nc.gpsimd.partition_broadcast(bcf[:], rcr[:])
                            nc.vector.tensor_mul(
                                atts[qb_t][pr][sub * 64:(sub + 1) * 64, :],
                                u2[sub * 64:(sub + 1) * 64, :],
                                bcf[sub * 64:(sub + 1) * 64, :])
                        return emit

                    for u2, pr in ((cur["uA_sb"], pA), (ub_sb, pB)):
                        for sub in range(2):
                            norm_q.append(norm_sub(u2, pr, sub))

        # driver
        from collections import deque
        norm_q = deque()
        fidx = 0            # index into filler list
        fchunk = 0          # next chunk within filler[fidx]
        next_u = 0          # next deferred-U slot to emit
        MM_NS = 270.0
        PAIR_NS = 250.0
        BUDGET = 980.0 - PAIR_NS   # per-slot non-scores PE budget

        def u_ready(j, i):
            if j >= min(i, nsteps):
                return False
            kt = step_list[j][4]
            return kt in vdone

        pend_sc = emit_scores(*[x for x in
                                (step_list[0][0], step_list[0][1],
                                 step_list[0][2], step_list[0][4])])

        slot_mms = []
        for i in range(nsteps):
            # exp for slot i
            et = ex_p.tile([128, 2 * QB], BF16, tag="ex", name="ex")
            nc.scalar.activation(
                et[:], pend_sc[:],
                mybir.ActivationFunctionType.Exp, scale=0.125)
            ets[i] = et
            # scores for slot i+1
            if i + 1 < nsteps:
                nq, nqb, npair, _, nkt = step_list[i + 1]
                pend_sc = emit_scores(nq, nqb, npair, nkt)
            nmm = 0
            budget = BUDGET
            if norm_q:
                norm_q.popleft()()

            def pop_filler():
                nonlocal fidx, fchunk, vemitted, nmm
                dl, vkt, chunks, earliest, u_req = filler[fidx]
                chunks[fchunk]()
                nmm += 2
                fchunk += 1
                if fchunk == len(chunks):
                    if vkt is not None:
                        vdone.add(vkt)
                    fidx += 1
                    fchunk = 0

            # 1. overdue filler chunks (deadline at this slot)
            while fidx < len(filler):
                dl = filler[fidx][0]
                if dl is not None and dl <= i + 1:
                    pop_filler()
                    budget -= MM_NS * 2
                else:
                    break

            # 2. et-ring pressure: force U so exp(i+NB-2) won't stall
            while next_u <= i and i - next_u >= NB - 4 and u_ready(next_u, i):
                emit_u(next_u)
                next_u += 1
                nmm += 2
                budget -= PAIR_NS
            # 3. budgeted: prefer U (keeps ring drained), then fillers
            while budget > 0:
                if u_ready(next_u, i):
                    emit_u(next_u)
                    next_u += 1
                    nmm += 2
                    budget -= PAIR_NS
                elif (fidx < len(filler) and filler[fidx][3] <= i
                      and filler[fidx][4] <= next_u
                      and (filler[fidx][4] == 0 or not norm_q)):
                    pop_filler()
                    budget -= MM_NS * 2
                else:
                    break
            slot_mms.append(nmm)

        # drain remaining U + fillers (tail)
        while next_u < nsteps:
            emit_u(next_u)
            next_u += 1
        while norm_q:
            norm_q.popleft()()
        while fidx < len(filler):
            dl, vkt, chunks, earliest, u_req = filler[fidx]
            for c in range(fchunk, len(chunks)):
                chunks[c]()
            if vkt is not None:
                vdone.add(vkt)
            fidx += 1
            fchunk = 0
        # final: og3b
        for eo in range(NET):
            og3b_chunk(eo)()

        if "--slotdump" in sys.argv:
            print("slot mm loads:", slot_mms)

    nc.compile()
    return nc


_CACHED = {}


def _get_program():
    if "nc" not in _CACHED:
        _CACHED["nc"] = build_program()
    return _CACHED["nc"]


def make_inputs(embeddings, wq, bq, wk, bk, wv, bv, wo, bo):
    in_maps = []
    for c in range(N_CORES):
        b, half = c // 2, c % 2
        sl = slice(half * OL, (half + 1) * OL)
        in_maps.append({
            "xT": np.ascontiguousarray(embeddings[b].T).astype(NPBF16),
            "wqT": np.ascontiguousarray(wq[sl, :].T).astype(NPBF16),
            "wkT": np.ascontiguousarray(wk[sl, :].T).astype(NPBF16),
            "wvT": np.ascontiguousarray(wv[sl, :].T).astype(NPBF16),
            "woT": np.ascontiguousarray(wo[:, sl].T).astype(NPBF16),
            "bqc": np.ascontiguousarray(
                bq[sl].reshape(4, 128).T).astype(np.float32),
            "bkc": np.ascontiguousarray(
                bk[sl].reshape(4, 128).T).astype(np.float32),
        })
    return in_maps


def unshard(results, bo):
    out = np.empty((B, S, E), np.float32)
    for b in range(B):
        yt = results[2 * b]["yT"] + results[2 * b + 1]["yT"]
        out[b] = yt.T + bo[None, :]
    return out


def fold_bv(bv, wo, bo):
    # softmax rows sum to 1, so V's bias contributes the constant bv @ wo.T
    return bo + bv.astype(np.float32) @ wo.astype(np.float32).T


def kernel(embeddings, wq, bq, wk, bk, wv, bv, wo, bo, _trace=False):
    embeddings = np.asarray(embeddings, np.float32)
    nc = _get_program()
    in_maps = make_inputs(
        embeddings, np.asarray(wq, np.float32), np.asarray(bq, np.float32),
        np.asarray(wk, np.float32), np.asarray(bk, np.float32),
        np.asarray(wv, np.float32), np.asarray(bv, np.float32),
        np.asarray(wo, np.float32), np.asarray(bo, np.float32))
    res = run_bass_kernel_spmd(
        nc, in_maps, core_ids=list(range(N_CORES)), trace=_trace)
    bo_eff = fold_bv(np.asarray(bv, np.float32), np.asarray(wo, np.float32),
                     np.asarray(bo, np.float32))
    out = unshard(res.results, bo_eff)
    if _trace:
        kernel.last_result = res
    return out
